# revision 9
# baseline (speedup 1.0000x reference)
"""Bass/Trainium2 kernel for nn_MultiHeadAttentionBlock_23502061043960.

Reference math (note: the module multiplies RAW scores with value — no
softmax in the output path — so the whole block is linear):

    out = (concat_h Q_h (K_h^T V_h) / 8) @ w_o.T + b_o
        where Q = q w_q^T, K = k w_k^T, V = v w_v^T   (biases are zero)

Linearity lets us contract the sequence dim first and never materialize
the [B,H,S,S] score tensor:

    A_b    = k_b^T v_b                     [512, 512]   (per batch)
    M_h    = w_k[h] A_b w_v[h]^T / 8       [64, 64]     (per head)
    W2     = w_o blockdiag(M_h^T)          [512, 512]
    out_b  = q_b w_q^T W2^T + b_o

Sharding over 8 cores: core c owns batch c//4 and sequence-quarter c%4.
Phase 1 computes the partial A over the core's 1024 k/v rows and folds
it down to the per-head M^T blocks locally (the fold is linear, so it
commutes with the cross-core row sum); an AllGather of the 128 KiB M^T
partials within each 4-core batch group + a local sum completes M.
Phase 2 folds M into W2 and applies it to the core's own q rows.

q is staged host-side as q^T (and the output is returned as out^T)
because the PE array contracts over the partition dim; weights are
staged as W^T so they can be the stationary operand directly. All
matmul inputs are float32r (full-rate fp32 mode on the PE; ~1e-4 rel
error vs the fp32 reference, far inside the 2e-2 gate).
"""

import numpy as np

import concourse.bass as bass
import concourse.mybir as mybir
import concourse.tile as tile
from concourse import bacc
from concourse.bass_utils import run_bass_kernel_spmd

B = 2
S = 4096
D = 512
H = 8
DK = 64
N_CORES = 8
SQ = S // 4  # 1024 sequence rows per core
P = 128
F32 = mybir.dt.float32

USE_F32R = True

_compiled = {}

LAST_RESULTS = None  # test harness reads exec_time_ns / trace from here
RUN_KW = {}  # test harness can inject trace kwargs


def _build():
    nc = bacc.Bacc()

    DT_MM = mybir.dt.float32r if USE_F32R else F32

    kq = nc.declare_dram_parameter("kq", [SQ, D], DT_MM, isOutput=False)
    vq = nc.declare_dram_parameter("vq", [SQ, D], DT_MM, isOutput=False)
    qT = nc.declare_dram_parameter("qT", [D, SQ], DT_MM, isOutput=False)
    wkT = nc.declare_dram_parameter("wkT", [D, D], DT_MM, isOutput=False)
    wvT = nc.declare_dram_parameter("wvT", [D, D], DT_MM, isOutput=False)
    wqT = nc.declare_dram_parameter("wqT", [D, D], DT_MM, isOutput=False)
    woT = nc.declare_dram_parameter("woT", [D, D], DT_MM, isOutput=False)
    bo = nc.declare_dram_parameter("bo", [P, 4], F32, isOutput=False)
    outT = nc.declare_dram_parameter("outT", [D, SQ], F32, isOutput=True)

    kq_v = kq.rearrange("(n p) d -> n p d", p=P)  # 8 x [128, 512]
    vq_v = vq.rearrange("(n p) d -> n p d", p=P)
    qT_v = qT.rearrange("(n p) d -> n p d", p=P)  # 4 x [128, 1024]
    wkT_v = wkT.rearrange("(n p) d -> n p d", p=P)  # 4 x [128, 512]
    wvT_v = wvT.rearrange("(n p) d -> n p d", p=P)
    wqT_v = wqT.rearrange("(n p) d -> n p d", p=P)
    woT_v = woT.rearrange("(n p) d -> n p d", p=P)
    outT_v = outT.rearrange("(n p) (m d) -> n m p d", p=P, d=D)  # [4,2,128,512]

    NKC = SQ // P  # 8 contraction chunks for A
    NDC = D // P  # 4 chunks of the model dim
    NN = SQ // D  # 2 column chunks of the row dim

    with tile.TileContext(nc) as tc:
        with (
            tc.tile_pool(name="w", bufs=1) as wp,
            tc.tile_pool(name="kv", bufs=1) as kvp,
            tc.tile_pool(name="qt", bufs=1) as qtp,
            tc.tile_pool(name="work", bufs=NDC) as wkpool,
            tc.tile_pool(name="big", bufs=NDC) as bigp,
            tc.tile_pool(name="small", bufs=1) as smallp,
            tc.tile_pool(name="ot", bufs=4) as otp,
            tc.tile_pool(name="psA", bufs=NDC, space="PSUM") as psa,
            tc.tile_pool(name="psB", bufs=4, space="PSUM") as psb,
            tc.tile_pool(name="dram", bufs=1, space="DRAM") as dramp,
        ):
            # ---- streaming loads: k/v first (phase 1 needs them first) ---
            k_t = [kvp.tile([P, D], DT_MM, name=f"k{i}", tag=f"k{i}") for i in range(NKC)]
            v_t = [kvp.tile([P, D], DT_MM, name=f"v{i}", tag=f"v{i}") for i in range(NKC)]
            for i in range(NKC):
                nc.sync.dma_start(out=k_t[i][:], in_=kq_v[i])
                nc.sync.dma_start(out=v_t[i][:], in_=vq_v[i])
            wk_t = [wp.tile([P, D], DT_MM, name=f"wk{i}", tag=f"wk{i}") for i in range(NDC)]
            wv_t = [wp.tile([P, D], DT_MM, name=f"wv{i}", tag=f"wv{i}") for i in range(NDC)]
            for i in range(NDC):
                nc.sync.dma_start(out=wk_t[i][:], in_=wkT_v[i])
                nc.sync.dma_start(out=wv_t[i][:], in_=wvT_v[i])
            qt_t = [qtp.tile([P, SQ], DT_MM, name=f"q{i}", tag=f"q{i}") for i in range(NDC)]
            wq_t = [wp.tile([P, D], DT_MM, name=f"wq{i}", tag=f"wq{i}") for i in range(NDC)]
            wo_t = [wp.tile([P, D], DT_MM, name=f"wo{i}", tag=f"wo{i}") for i in range(NDC)]
            for i in range(NDC):
                nc.sync.dma_start(out=qt_t[i][:], in_=qT_v[i])
                nc.sync.dma_start(out=wq_t[i][:], in_=wqT_v[i])
                nc.sync.dma_start(out=wo_t[i][:], in_=woT_v[i])
            bo_t = wp.tile([P, 4], F32, name="bo", tag="bo")
            nc.sync.dma_start(out=bo_t[:], in_=bo[:])

            # ---- phase 1: A = k^T v, kc-outer so PE consumes the DMA stream
            a_ps = [psa.tile([P, D], F32, name=f"aps{m}", tag="aps") for m in range(NDC)]
            for kc in range(NKC):
                for m in range(NDC):
                    nc.tensor.matmul(
                        a_ps[m][:],
                        k_t[kc][:, m * P : (m + 1) * P],
                        v_t[kc][:],
                        start=(kc == 0),
                        stop=(kc == NKC - 1),
                    )
            a_sb = []
            for m in range(NDC):
                t = wkpool.tile([P, D], DT_MM, name="a", tag="a")
                nc.vector.tensor_copy(t[:], a_ps[m][:])
                a_sb.append(t)

            # ---- fold F1: Y^T = A^T wkT  (Y = w_k A) ---------------------
            yT_sb = []
            for m in range(NDC):
                y_ps = psb.tile([P, D], F32, name="yps", tag="ps")
                for kc in range(NDC):
                    nc.tensor.matmul(
                        y_ps[:],
                        a_sb[kc][:, m * P : (m + 1) * P],
                        wk_t[kc][:],
                        start=(kc == 0),
                        stop=(kc == NDC - 1),
                    )
                t = wkpool.tile([P, D], DT_MM, name="yT", tag="yT")
                nc.vector.tensor_copy(t[:], y_ps[:])
                yT_sb.append(t)

            # ---- fold F2: diagonal band of G = w_v Y^T -------------------
            # G[64h+j, 64h+i] = M_h[i,j], so the diag blocks are M_h^T.
            m_loc = smallp.tile([DK, D], F32, name="mloc", tag="mloc")
            for m in range(NDC):
                g_ps = psb.tile([P, P], F32, name="gps", tag="ps")
                for kc in range(NDC):
                    nc.tensor.matmul(
                        g_ps[:],
                        wv_t[kc][:, m * P : (m + 1) * P],
                        yT_sb[kc][:, m * P : (m + 1) * P],
                        start=(kc == 0),
                        stop=(kc == NDC - 1),
                    )
                for hh in range(2):  # heads 2m, 2m+1
                    h = 2 * m + hh
                    # fold the 1/sqrt(dk)=1/8 score scale into M
                    nc.scalar.mul(
                        m_loc[:, h * DK : (h + 1) * DK],
                        g_ps[hh * DK : (hh + 1) * DK, hh * DK : (hh + 1) * DK],
                        0.125,
                    )

            # ---- AllGather M^T partials within the 4-core batch group ----
            m_in = dramp.tile([DK, D], F32, name="min", tag="min")
            m_gath = dramp.tile([4 * DK, D], F32, name="mgath", tag="mgath")
            nc.sync.dma_start(out=m_in[:], in_=m_loc[:])
            nc.gpsimd.collective_compute(
                "AllGather",
                mybir.AluOpType.bypass,
                replica_groups=[[0, 1, 2, 3], [4, 5, 6, 7]],
                ins=[m_in.opt()],
                outs=[m_gath.opt()],
            )
            # load the 4 gathered partials side by side on the same
            # partitions: gt[p, g*512+d] = m_gath[g*64+p, d]
            gt = smallp.tile([DK, 4 * D], F32, name="gt", tag="gt")
            nc.sync.dma_start(
                out=gt[:], in_=m_gath.rearrange("(g p) d -> p g d", p=DK)
            )
            s0 = smallp.tile([DK, D], F32, name="s0", tag="s0")
            s1 = smallp.tile([DK, D], F32, name="s1", tag="s1")
            m_red = smallp.tile([DK, D], DT_MM, name="mred", tag="mred")
            nc.vector.tensor_add(s0[:], gt[:, 0:D], gt[:, D : 2 * D])
            nc.vector.tensor_add(s1[:], gt[:, 2 * D : 3 * D], gt[:, 3 * D : 4 * D])
            nc.vector.tensor_add(m_red[:], s0[:], s1[:])

            # ---- phase 2a: Qp^T = w_q q^T (independent of the collective)
            qp_sb = [bigp.tile([P, SQ], DT_MM, name=f"qp{m}", tag="qp") for m in range(NDC)]
            for m in range(NDC):
                for nn in range(NN):
                    ns = slice(nn * D, (nn + 1) * D)
                    q_ps = psb.tile([P, D], F32, name="qps", tag="ps")
                    for kc in range(NDC):
                        nc.tensor.matmul(
                            q_ps[:],
                            wq_t[kc][:, m * P : (m + 1) * P],
                            qt_t[kc][:, ns],
                            start=(kc == 0),
                            stop=(kc == NDC - 1),
                        )
                    nc.vector.tensor_copy(qp_sb[m][:, ns], q_ps[:])

            # ---- phase 2b: W2^T = BD(M) woT  (W2 = w_o BD(M)^T) ----------
            # bd[p] = blockdiag(M_2p^T, M_2p+1^T); quadrants written with
            # SBUF->SBUF DMAs (partition-offset writes are not lane-bound).
            w2_sb = []
            for p in range(NDC):
                bd = smallp.tile([P, P], DT_MM, name=f"bd{p}", tag=f"bd{p}")
                nc.gpsimd.memset(bd[:].bitcast(mybir.dt.uint32), 0)
                nc.sync.dma_start(
                    out=bd[0:DK, 0:DK], in_=m_red[:, (2 * p) * DK : (2 * p + 1) * DK]
                )
                nc.sync.dma_start(
                    out=bd[DK:P, DK:P], in_=m_red[:, (2 * p + 1) * DK : (2 * p + 2) * DK]
                )
                w2_ps = psb.tile([P, D], F32, name="w2ps", tag="ps")
                nc.tensor.matmul(w2_ps[:], bd[:], wo_t[p][:], start=True, stop=True)
                t = wkpool.tile([P, D], DT_MM, name="w2", tag="w2")
                nc.vector.tensor_copy(t[:], w2_ps[:])
                w2_sb.append(t)

            # ---- phase 2c: out^T = W2 Qp^T + b_o -------------------------
            for m in range(NDC):
                for nn in range(NN):
                    ns = slice(nn * D, (nn + 1) * D)
                    o_ps = psb.tile([P, D], F32, name="ops", tag="ps")
                    for kc in range(NDC):
                        nc.tensor.matmul(
                            o_ps[:],
                            w2_sb[kc][:, m * P : (m + 1) * P],
                            qp_sb[kc][:, ns],
                            start=(kc == 0),
                            stop=(kc == NDC - 1),
                        )
                    o_sb = otp.tile([P, D], F32, name="osb", tag="osb")
                    nc.scalar.activation(
                        o_sb[:],
                        o_ps[:],
                        mybir.ActivationFunctionType.Identity,
                        bias=bo_t[:, m : m + 1],
                    )
                    nc.sync.dma_start(out=outT_v[m, nn], in_=o_sb[:])

    nc.compile()
    return nc


def kernel(q, k, v, w_q, b_q, w_k, b_k, w_v, b_v, w_o, b_o):
    global LAST_RESULTS
    key = ("nc", USE_F32R)
    if key not in _compiled:
        _compiled[key] = _build()
    nc = _compiled[key]

    q = np.asarray(q, dtype=np.float32)
    k = np.asarray(k, dtype=np.float32)
    v = np.asarray(v, dtype=np.float32)
    wkT = np.ascontiguousarray(np.asarray(w_k, np.float32).T)
    wvT = np.ascontiguousarray(np.asarray(w_v, np.float32).T)
    wqT = np.ascontiguousarray(np.asarray(w_q, np.float32).T)
    woT = np.ascontiguousarray(np.asarray(w_o, np.float32).T)
    bo = np.ascontiguousarray(np.asarray(b_o, np.float32).reshape(4, P).T)

    in_maps = []
    for c in range(N_CORES):
        b, quarter = divmod(c, 4)
        rows = slice(quarter * SQ, (quarter + 1) * SQ)
        in_maps.append(
            {
                "kq": np.ascontiguousarray(k[b, rows, :]),
                "vq": np.ascontiguousarray(v[b, rows, :]),
                "qT": np.ascontiguousarray(q[b, rows, :].T),
                "wkT": wkT,
                "wvT": wvT,
                "wqT": wqT,
                "woT": woT,
                "bo": bo,
            }
        )

    res = run_bass_kernel_spmd(nc, in_maps, list(range(N_CORES)), **RUN_KW)
    LAST_RESULTS = res

    out = np.empty((B, S, D), dtype=np.float32)
    for c in range(N_CORES):
        b, quarter = divmod(c, 4)
        rows = slice(quarter * SQ, (quarter + 1) * SQ)
        out[b, rows, :] = res.results[c]["outT"].T
    return out


# revision 13
# speedup vs baseline: 1.7430x; 1.7430x over previous
"""Bass/Trainium2 kernel for nn_MultiHeadAttentionBlock_23502061043960.

Reference math (note: the module multiplies RAW scores with value — no
softmax in the output path — so the whole block is linear):

    out = (concat_h Q_h (K_h^T V_h) / 8) @ w_o.T + b_o
        where Q = q w_q^T, K = k w_k^T, V = v w_v^T   (biases are zero)

Linearity lets us contract the sequence dim first and never materialize
the [B,H,S,S] score tensor:

    A_b    = k_b^T v_b                     [512, 512]   (per batch)
    M_h    = w_k[h] A_b w_v[h]^T / 8       [64, 64]     (per head)
    W2     = w_o blockdiag(M_h^T)          [512, 512]
    out_b  = q_b w_q^T W2^T + b_o

Sharding over 8 cores: core c owns batch c//4 and sequence-quarter c%4
of the output rows. Each core computes the full A_b from the full
k_b/v_b (4x redundant, but collective-free: on this stack a collective
drags in an all-core start barrier that costs far more than the extra
DMA), folds it to W2, and applies it to its own q rows.

q is staged host-side as q^T (and the output is returned as out^T)
because the PE array contracts over the partition dim; weights are
staged as W^T so they can be the stationary operand directly.

dtype: matmul inputs are bf16 (staged host-side), fp32 PSUM
accumulation throughout; measured rel err vs the fp32 reference is
~1e-3, far inside the 2e-2 gate, and it halves both DMA bytes and PE
cycles vs fp32. Set USE_BF16=False for an fp32r build (~3e-4).
"""

import ml_dtypes
import numpy as np

import concourse.bass as bass
import concourse.mybir as mybir
import concourse.tile as tile
from concourse import bacc
from concourse.bass_utils import run_bass_kernel_spmd

B = 2
S = 4096
D = 512
H = 8
DK = 64
N_CORES = 8
SQ = S // 4  # 1024 output rows per core
P = 128
F32 = mybir.dt.float32

USE_BF16 = True

_compiled = {}

LAST_RESULTS = None  # test harness reads exec_time_ns / trace from here
RUN_KW = {}  # test harness can inject trace kwargs


def _build():
    nc = bacc.Bacc()

    DT = mybir.dt.bfloat16 if USE_BF16 else mybir.dt.float32r

    # k/v and weights are host-staged "quad-packed": 4 row-chunks of
    # [128, 512] side by side in one [128, 2048] tile, so every DMA
    # destination partition row is a 2 KiB contiguous DRAM run (bf16
    # at the natural [row, 512] layout only gives 1 KiB runs, which
    # halves effective DMA bandwidth).
    kb = nc.declare_dram_parameter("kb", [S // 4, 4 * D], DT, isOutput=False)
    vb = nc.declare_dram_parameter("vb", [S // 4, 4 * D], DT, isOutput=False)
    qT = nc.declare_dram_parameter("qT", [D, SQ], DT, isOutput=False)
    wkT = nc.declare_dram_parameter("wkT", [P, 4 * D], DT, isOutput=False)
    wvT = nc.declare_dram_parameter("wvT", [P, 4 * D], DT, isOutput=False)
    wq = nc.declare_dram_parameter("wq", [P, 4 * D], DT, isOutput=False)
    woT = nc.declare_dram_parameter("woT", [P, 4 * D], DT, isOutput=False)
    bo = nc.declare_dram_parameter("bo", [P, 4], F32, isOutput=False)
    outT = nc.declare_dram_parameter("outT", [D, SQ], DT, isOutput=True)

    kb_v = kb.rearrange("(n p) d -> n p d", p=P)  # 8 x [128, 2048]
    vb_v = vb.rearrange("(n p) d -> n p d", p=P)
    qT_v = qT.rearrange("(n p) d -> n p d", p=P)  # 4 x [128, 1024]
    outT_v = outT.rearrange("(n p) d -> n p d", p=P)  # 4 x [128, 1024]

    NKC = S // P  # 32 contraction chunks for A
    NDC = D // P  # 4 chunks of the model dim
    NG = NKC // 4  # 8 quad-packed k/v tiles

    with tile.TileContext(nc) as tc:
        with (
            tc.tile_pool(name="w", bufs=1) as wp,
            tc.tile_pool(name="kv", bufs=1) as kvp,
            tc.tile_pool(name="qt", bufs=1) as qtp,
            tc.tile_pool(name="work", bufs=NDC) as wkpool,
            tc.tile_pool(name="big", bufs=NDC) as bigp,
            tc.tile_pool(name="small", bufs=1) as smallp,
            tc.tile_pool(name="ot", bufs=8) as otp,
            tc.tile_pool(name="psB", bufs=4, space="PSUM") as psb,
        ):
            # psA lives only for phase 1; closing it lets psW reuse its
            # banks (PSUM is 8 banks total: 4 psA / 2 psB / 4 psW).
            a_sb = []
            with tc.tile_pool(name="psA", bufs=NDC, space="PSUM") as psa:
                # ---- phase 1: A = k^T v, streaming k/v chunk pairs -------
                # loads and matmuls interleaved: the PE chases the DMA stream
                a_ps = [psa.tile([P, D], F32, name=f"aps{m}", tag="aps") for m in range(NDC)]
                # first quad as 4 standalone chunk tiles so the very first
                # matmul only waits on a 0.25 MiB pair, not the full quad
                k0 = [kvp.tile([P, D], DT, name=f"k0{j}", tag=f"k0{j}") for j in range(4)]
                v0 = [kvp.tile([P, D], DT, name=f"v0{j}", tag=f"v0{j}") for j in range(4)]
                k_t = [kvp.tile([P, 4 * D], DT, name=f"k{i}", tag=f"k{i}") for i in range(1, NG)]
                v_t = [kvp.tile([P, 4 * D], DT, name=f"v{i}", tag=f"v{i}") for i in range(1, NG)]
                for j in range(4):
                    js = slice(j * D, (j + 1) * D)
                    nc.sync.dma_start(out=k0[j][:], in_=kb_v[0][:, js])
                    nc.sync.dma_start(out=v0[j][:], in_=vb_v[0][:, js])
                    for m in range(NDC):
                        nc.tensor.matmul(
                            a_ps[m][:],
                            k0[j][:, m * P : (m + 1) * P],
                            v0[j][:],
                            start=(j == 0),
                            stop=False,
                        )
                for g in range(1, NG):
                    nc.sync.dma_start(out=k_t[g - 1][:], in_=kb_v[g])
                    nc.sync.dma_start(out=v_t[g - 1][:], in_=vb_v[g])
                    for j in range(4):
                        for m in range(NDC):
                            nc.tensor.matmul(
                                a_ps[m][:],
                                k_t[g - 1][:, j * D + m * P : j * D + (m + 1) * P],
                                v_t[g - 1][:, j * D : (j + 1) * D],
                                start=False,
                                stop=(g == NG - 1 and j == 3),
                            )

                # ---- remaining loads on the gpsimd DMA queue so they
                # stream concurrently with the k/v stream on sync ----------
                # after the k/v stream on the same ring: they land during
                # A's PE-bound phase without stealing HBM bandwidth earlier
                wk_t = wp.tile([P, 4 * D], DT, name="wkt", tag="wkt")
                wv_t = wp.tile([P, 4 * D], DT, name="wvt", tag="wvt")
                wq_t = wp.tile([P, 4 * D], DT, name="wqt", tag="wqt")
                wo_t = wp.tile([P, 4 * D], DT, name="wot", tag="wot")
                nc.sync.dma_start(out=wk_t[:], in_=wkT[:])
                nc.sync.dma_start(out=wv_t[:], in_=wvT[:])
                qt_t = [qtp.tile([P, SQ], DT, name=f"q{i}", tag=f"q{i}") for i in range(NDC)]
                for i in range(NDC):
                    nc.sync.dma_start(out=qt_t[i][:], in_=qT_v[i])
                nc.sync.dma_start(out=wq_t[:], in_=wq[:])
                nc.sync.dma_start(out=wo_t[:], in_=woT[:])
                bo_t = wp.tile([P, 4], F32, name="bo", tag="bo")
                nc.sync.dma_start(out=bo_t[:], in_=bo[:])

                for m in range(NDC):
                    t = wkpool.tile([P, D], DT, name="a", tag="a")
                    nc.vector.tensor_copy(t[:], a_ps[m][:])
                    a_sb.append(t)

            with tc.tile_pool(name="psW", bufs=4, space="PSUM") as psw:
                # ---- fold F1: Y^T = A^T wkT  (Y = w_k A) ---------------------
                yT_sb = []
                for m in range(NDC):
                    y_ps = psb.tile([P, D], F32, name="yps", tag="ps")
                    for kc in range(NDC):
                        nc.tensor.matmul(
                            y_ps[:],
                            a_sb[kc][:, m * P : (m + 1) * P],
                            wk_t[:, kc * D : (kc + 1) * D],
                            start=(kc == 0),
                            stop=(kc == NDC - 1),
                        )
                    t = wkpool.tile([P, D], DT, name="yT", tag="yT")
                    nc.vector.tensor_copy(t[:], y_ps[:])
                    yT_sb.append(t)

                # ---- fold F2: diagonal band of G = w_v Y^T -------------------
                # G[64h+j, 64h+i] = M_h[i,j], so the diag blocks are M_h^T.
                m_loc = smallp.tile([DK, D], DT, name="mloc", tag="mloc")
                for m in range(NDC):
                    g_ps = psb.tile([P, P], F32, name="gps", tag="ps")
                    for kc in range(NDC):
                        nc.tensor.matmul(
                            g_ps[:],
                            wv_t[:, kc * D + m * P : kc * D + (m + 1) * P],
                            yT_sb[kc][:, m * P : (m + 1) * P],
                            start=(kc == 0),
                            stop=(kc == NDC - 1),
                        )
                    for hh in range(2):  # heads 2m, 2m+1
                        h = 2 * m + hh
                        # (the 1/sqrt(dk)=1/8 score scale is folded into the
                        # host-staged wkT)
                        nc.scalar.copy(
                            m_loc[:, h * DK : (h + 1) * DK],
                            g_ps[hh * DK : (hh + 1) * DK, hh * DK : (hh + 1) * DK],
                        )

                # ---- phase 2b: W2^T = BD(M) woT  (W2 = w_o BD(M)^T) ----------
                # bd[p] = blockdiag(M_2p^T, M_2p+1^T); quadrants written with
                # SBUF->SBUF DMAs (partition-offset writes are not lane-bound).
                w2_sb = []
                for p in range(NDC):
                    bd = smallp.tile([P, P], DT, name=f"bd{p}", tag=f"bd{p}")
                    nc.gpsimd.memset(bd[:].bitcast(mybir.dt.uint32), 0)
                    nc.sync.dma_start(
                        out=bd[0:DK, 0:DK], in_=m_loc[:, (2 * p) * DK : (2 * p + 1) * DK]
                    )
                    nc.sync.dma_start(
                        out=bd[DK:P, DK:P], in_=m_loc[:, (2 * p + 1) * DK : (2 * p + 2) * DK]
                    )
                    w2_ps = psb.tile([P, D], F32, name="w2ps", tag="ps")
                    nc.tensor.matmul(w2_ps[:], bd[:], wo_t[:, p * D : (p + 1) * D], start=True, stop=True)
                    t = wkpool.tile([P, D], DT, name="w2", tag="w2")
                    nc.vector.tensor_copy(t[:], w2_ps[:])
                    w2_sb.append(t)

                # ---- fold Wfold = w_q^T W2^T  (out = q Wfold + b_o) ----------
                wf_sb = []
                for m in range(NDC):
                    wf_ps = psb.tile([P, D], F32, name="wfps", tag="ps")
                    for kc in range(NDC):
                        nc.tensor.matmul(
                            wf_ps[:],
                            wq_t[:, kc * D + m * P : kc * D + (m + 1) * P],
                            w2_sb[kc][:],
                            start=(kc == 0),
                            stop=(kc == NDC - 1),
                        )
                    t = wkpool.tile([P, D], DT, name="wf", tag="wf")
                    nc.vector.tensor_copy(t[:], wf_ps[:])
                    wf_sb.append(t)

                # ---- phase 2c: out^T = W2 Qp^T + b_o -------------------------
                for m in range(NDC):
                    for nn in range(SQ // D):
                        ns = slice(nn * D, (nn + 1) * D)
                        o_ps = psw.tile([P, D], F32, name="ops", tag="pw")
                        for kc in range(NDC):
                            nc.tensor.matmul(
                                o_ps[:],
                                wf_sb[kc][:, m * P : (m + 1) * P],
                                qt_t[kc][:, ns],
                                start=(kc == 0),
                                stop=(kc == NDC - 1),
                            )
                        o_sb = otp.tile([P, D], DT, name="osb", tag="osb")
                        nc.vector.tensor_scalar_add(o_sb[:], o_ps[:], bo_t[:, m : m + 1])
                        nc.sync.dma_start(out=outT_v[m][:, ns], in_=o_sb[:])

    nc.compile()
    return nc


def kernel(q, k, v, w_q, b_q, w_k, b_k, w_v, b_v, w_o, b_o):
    global LAST_RESULTS
    key = ("nc", USE_BF16)
    if key not in _compiled:
        _compiled[key] = _build()
    nc = _compiled[key]

    np_dt = ml_dtypes.bfloat16 if USE_BF16 else np.float32

    def pack4(x):  # [N, 512] -> [N//4, 2048]: 4 row-chunks side by side
        n = x.shape[0] // (4 * P)
        return np.ascontiguousarray(
            x.reshape(n, 4, P, D).transpose(0, 2, 1, 3).reshape(n * P, 4 * D)
        )

    q = np.asarray(q, dtype=np.float32)
    kc_ = [pack4(np.asarray(k[b], np.float32).astype(np_dt)) for b in range(B)]
    vc_ = [pack4(np.asarray(v[b], np.float32).astype(np_dt)) for b in range(B)]
    wkT = pack4((np.asarray(w_k, np.float32).T * 0.125).astype(np_dt))
    wvT = pack4(np.asarray(w_v, np.float32).T.astype(np_dt))
    wqn = pack4(np.asarray(w_q, np.float32).astype(np_dt))
    woT = pack4(np.asarray(w_o, np.float32).T.astype(np_dt))
    bo = np.ascontiguousarray(np.asarray(b_o, np.float32).reshape(4, P).T)

    in_maps = []
    for c in range(N_CORES):
        b, quarter = divmod(c, 4)
        rows = slice(quarter * SQ, (quarter + 1) * SQ)
        in_maps.append(
            {
                "kb": kc_[b],
                "vb": vc_[b],
                "qT": np.ascontiguousarray(q[b, rows, :].T).astype(np_dt),
                "wkT": wkT,
                "wvT": wvT,
                "wq": wqn,
                "woT": woT,
                "bo": bo,
            }
        )

    res = run_bass_kernel_spmd(nc, in_maps, list(range(N_CORES)), **RUN_KW)
    LAST_RESULTS = res

    out = np.empty((B, S, D), dtype=np.float32)
    for c in range(N_CORES):
        b, quarter = divmod(c, 4)
        rows = slice(quarter * SQ, (quarter + 1) * SQ)
        out[b, rows, :] = res.results[c]["outT"].T.astype(np.float32)
    return out



# revision 14
# speedup vs baseline: 1.8350x; 1.0528x over previous
"""Bass/Trainium2 kernel for nn_MultiHeadAttentionBlock_23502061043960.

Reference math (note: the module multiplies RAW scores with value — no
softmax in the output path — so the whole block is linear):

    out = (concat_h Q_h (K_h^T V_h) / 8) @ w_o.T + b_o
        where Q = q w_q^T, K = k w_k^T, V = v w_v^T   (biases are zero)

Linearity lets us contract the sequence dim first and never materialize
the [B,H,S,S] score tensor:

    A_b    = k_b^T v_b                     [512, 512]   (per batch)
    M_h    = w_k[h] A_b w_v[h]^T / 8       [64, 64]     (per head)
    W2     = w_o blockdiag(M_h^T)          [512, 512]
    out_b  = q_b w_q^T W2^T + b_o

Sharding over 8 cores: core c owns batch c//4 and sequence-quarter c%4
of the output rows. Each core computes the full A_b from the full
k_b/v_b (4x redundant, but collective-free: on this stack a collective
drags in an all-core start barrier that costs far more than the extra
DMA), folds it to W2, and applies it to its own q rows.

q is staged host-side as q^T (and the output is returned as out^T)
because the PE array contracts over the partition dim; weights are
staged as W^T so they can be the stationary operand directly.

dtype: matmul inputs are bf16 (staged host-side), fp32 PSUM
accumulation throughout; measured rel err vs the fp32 reference is
~1e-3, far inside the 2e-2 gate, and it halves both DMA bytes and PE
cycles vs fp32. Set USE_BF16=False for an fp32r build (~3e-4).
"""

import ml_dtypes
import numpy as np

import concourse.bass as bass
import concourse.mybir as mybir
import concourse.tile as tile
from concourse import bacc
from concourse.bass_utils import run_bass_kernel_spmd

B = 2
S = 4096
D = 512
H = 8
DK = 64
N_CORES = 8
SQ = S // 4  # 1024 output rows per core
P = 128
F32 = mybir.dt.float32

USE_BF16 = True

_compiled = {}

LAST_RESULTS = None  # test harness reads exec_time_ns / trace from here
RUN_KW = {}  # test harness can inject trace kwargs


def _build():
    nc = bacc.Bacc()

    DT = mybir.dt.bfloat16 if USE_BF16 else mybir.dt.float32r

    # k/v and weights are host-staged "quad-packed": 4 row-chunks of
    # [128, 512] side by side in one [128, 2048] tile, so every DMA
    # destination partition row is a 2 KiB contiguous DRAM run (bf16
    # at the natural [row, 512] layout only gives 1 KiB runs, which
    # halves effective DMA bandwidth).
    kb = nc.declare_dram_parameter("kb", [S // 2, 2 * D], DT, isOutput=False)
    vb = nc.declare_dram_parameter("vb", [S // 2, 2 * D], DT, isOutput=False)
    qT = nc.declare_dram_parameter("qT", [D, SQ], DT, isOutput=False)
    wkT = nc.declare_dram_parameter("wkT", [P, 4 * D], DT, isOutput=False)
    wvT = nc.declare_dram_parameter("wvT", [P, 4 * D], DT, isOutput=False)
    wq = nc.declare_dram_parameter("wq", [P, 4 * D], DT, isOutput=False)
    woT = nc.declare_dram_parameter("woT", [P, 4 * D], DT, isOutput=False)
    bo = nc.declare_dram_parameter("bo", [P, 4], F32, isOutput=False)
    outT = nc.declare_dram_parameter("outT", [D, SQ], DT, isOutput=True)

    kb_v = kb.rearrange("(n p) d -> n p d", p=P)  # 16 x [128, 1024]
    vb_v = vb.rearrange("(n p) d -> n p d", p=P)
    qT_v = qT.rearrange("(n p) d -> n p d", p=P)  # 4 x [128, 1024]
    outT_v = outT.rearrange("(n p) d -> n p d", p=P)  # 4 x [128, 1024]

    NKC = S // P  # 32 contraction chunks for A
    NDC = D // P  # 4 chunks of the model dim
    NG = NKC // 2  # 16 pair-packed k/v tiles

    with tile.TileContext(nc) as tc:
        with (
            tc.tile_pool(name="w", bufs=1) as wp,
            tc.tile_pool(name="kv", bufs=1) as kvp,
            tc.tile_pool(name="qt", bufs=1) as qtp,
            tc.tile_pool(name="work", bufs=NDC) as wkpool,
            tc.tile_pool(name="big", bufs=NDC) as bigp,
            tc.tile_pool(name="small", bufs=1) as smallp,
            tc.tile_pool(name="ot", bufs=8) as otp,
            tc.tile_pool(name="psB", bufs=4, space="PSUM") as psb,
        ):
            # psA lives only for phase 1; closing it lets psW reuse its
            # banks (PSUM is 8 banks total: 4 psA / 2 psB / 4 psW).
            a_sb = []
            with tc.tile_pool(name="psA", bufs=NDC, space="PSUM") as psa:
                # ---- phase 1: A = k^T v, streaming k/v chunk pairs -------
                # loads and matmuls interleaved: the PE chases the DMA stream
                a_ps = [psa.tile([P, D], F32, name=f"aps{m}", tag="aps") for m in range(NDC)]
                # first pair as 2 standalone chunk tiles so the very first
                # matmul only waits on a 0.25 MiB pair, not a full pair-tile
                k0 = [kvp.tile([P, D], DT, name=f"k0{j}", tag=f"k0{j}") for j in range(2)]
                v0 = [kvp.tile([P, D], DT, name=f"v0{j}", tag=f"v0{j}") for j in range(2)]
                k_t = [kvp.tile([P, 2 * D], DT, name=f"k{i}", tag=f"k{i}") for i in range(1, NG)]
                v_t = [kvp.tile([P, 2 * D], DT, name=f"v{i}", tag=f"v{i}") for i in range(1, NG)]
                for j in range(2):
                    js = slice(j * D, (j + 1) * D)
                    nc.sync.dma_start(out=k0[j][:], in_=kb_v[0][:, js])
                    nc.sync.dma_start(out=v0[j][:], in_=vb_v[0][:, js])
                    for m in range(NDC):
                        nc.tensor.matmul(
                            a_ps[m][:],
                            k0[j][:, m * P : (m + 1) * P],
                            v0[j][:],
                            start=(j == 0),
                            stop=False,
                        )
                for g in range(1, NG):
                    nc.sync.dma_start(out=k_t[g - 1][:], in_=kb_v[g])
                    nc.sync.dma_start(out=v_t[g - 1][:], in_=vb_v[g])
                    for j in range(2):
                        for m in range(NDC):
                            nc.tensor.matmul(
                                a_ps[m][:],
                                k_t[g - 1][:, j * D + m * P : j * D + (m + 1) * P],
                                v_t[g - 1][:, j * D : (j + 1) * D],
                                start=False,
                                stop=(g == NG - 1 and j == 1),
                            )

                # ---- remaining loads on the gpsimd DMA queue so they
                # stream concurrently with the k/v stream on sync ----------
                # after the k/v stream on the same ring: they land during
                # A's PE-bound phase without stealing HBM bandwidth earlier
                wk_t = wp.tile([P, 4 * D], DT, name="wkt", tag="wkt")
                wv_t = wp.tile([P, 4 * D], DT, name="wvt", tag="wvt")
                wq_t = wp.tile([P, 4 * D], DT, name="wqt", tag="wqt")
                wo_t = wp.tile([P, 4 * D], DT, name="wot", tag="wot")
                nc.sync.dma_start(out=wk_t[:], in_=wkT[:])
                nc.sync.dma_start(out=wv_t[:], in_=wvT[:])
                qt_t = [qtp.tile([P, SQ], DT, name=f"q{i}", tag=f"q{i}") for i in range(NDC)]
                for i in range(NDC):
                    nc.sync.dma_start(out=qt_t[i][:], in_=qT_v[i])
                nc.sync.dma_start(out=wo_t[:], in_=woT[:])
                nc.sync.dma_start(out=wq_t[:], in_=wq[:])
                bo_t = wp.tile([P, 4], F32, name="bo", tag="bo")
                nc.sync.dma_start(out=bo_t[:], in_=bo[:])

                for m in range(NDC):
                    t = wkpool.tile([P, D], DT, name="a", tag="a")
                    nc.vector.tensor_copy(t[:], a_ps[m][:])
                    a_sb.append(t)

            with tc.tile_pool(name="psW", bufs=4, space="PSUM") as psw:
                # ---- fold F1: Y^T = A^T wkT  (Y = w_k A) ---------------------
                yT_sb = []
                for m in range(NDC):
                    y_ps = psb.tile([P, D], F32, name="yps", tag="ps")
                    for kc in range(NDC):
                        nc.tensor.matmul(
                            y_ps[:],
                            a_sb[kc][:, m * P : (m + 1) * P],
                            wk_t[:, kc * D : (kc + 1) * D],
                            start=(kc == 0),
                            stop=(kc == NDC - 1),
                        )
                    t = wkpool.tile([P, D], DT, name="yT", tag="yT")
                    nc.vector.tensor_copy(t[:], y_ps[:])
                    yT_sb.append(t)

                # ---- fold F2: diagonal band of G = w_v Y^T -------------------
                # G[64h+j, 64h+i] = M_h[i,j], so the diag blocks are M_h^T.
                m_loc = smallp.tile([DK, D], DT, name="mloc", tag="mloc")
                for m in range(NDC):
                    g_ps = psb.tile([P, P], F32, name="gps", tag="ps")
                    for kc in range(NDC):
                        nc.tensor.matmul(
                            g_ps[:],
                            wv_t[:, kc * D + m * P : kc * D + (m + 1) * P],
                            yT_sb[kc][:, m * P : (m + 1) * P],
                            start=(kc == 0),
                            stop=(kc == NDC - 1),
                        )
                    for hh in range(2):  # heads 2m, 2m+1
                        h = 2 * m + hh
                        # (the 1/sqrt(dk)=1/8 score scale is folded into the
                        # host-staged wkT)
                        nc.scalar.copy(
                            m_loc[:, h * DK : (h + 1) * DK],
                            g_ps[hh * DK : (hh + 1) * DK, hh * DK : (hh + 1) * DK],
                        )

                # ---- phase 2b: W2^T = BD(M) woT  (W2 = w_o BD(M)^T) ----------
                # bd[p] = blockdiag(M_2p^T, M_2p+1^T); quadrants written with
                # SBUF->SBUF DMAs (partition-offset writes are not lane-bound).
                w2_sb = []
                for p in range(NDC):
                    bd = smallp.tile([P, P], DT, name=f"bd{p}", tag=f"bd{p}")
                    nc.gpsimd.memset(bd[:].bitcast(mybir.dt.uint32), 0)
                    nc.sync.dma_start(
                        out=bd[0:DK, 0:DK], in_=m_loc[:, (2 * p) * DK : (2 * p + 1) * DK]
                    )
                    nc.sync.dma_start(
                        out=bd[DK:P, DK:P], in_=m_loc[:, (2 * p + 1) * DK : (2 * p + 2) * DK]
                    )
                    w2_ps = psb.tile([P, D], F32, name="w2ps", tag="ps")
                    nc.tensor.matmul(w2_ps[:], bd[:], wo_t[:, p * D : (p + 1) * D], start=True, stop=True)
                    t = wkpool.tile([P, D], DT, name="w2", tag="w2")
                    nc.vector.tensor_copy(t[:], w2_ps[:])
                    w2_sb.append(t)

                # ---- fold Wfold = w_q^T W2^T  (out = q Wfold + b_o) ----------
                wf_sb = []
                for m in range(NDC):
                    wf_ps = psb.tile([P, D], F32, name="wfps", tag="ps")
                    for kc in range(NDC):
                        nc.tensor.matmul(
                            wf_ps[:],
                            wq_t[:, kc * D + m * P : kc * D + (m + 1) * P],
                            w2_sb[kc][:],
                            start=(kc == 0),
                            stop=(kc == NDC - 1),
                        )
                    t = wkpool.tile([P, D], DT, name="wf", tag="wf")
                    nc.vector.tensor_copy(t[:], wf_ps[:])
                    wf_sb.append(t)

                # ---- phase 2c: out^T = W2 Qp^T + b_o -------------------------
                for m in range(NDC):
                    for nn in range(SQ // D):
                        ns = slice(nn * D, (nn + 1) * D)
                        o_ps = psw.tile([P, D], F32, name="ops", tag="pw")
                        for kc in range(NDC):
                            nc.tensor.matmul(
                                o_ps[:],
                                wf_sb[kc][:, m * P : (m + 1) * P],
                                qt_t[kc][:, ns],
                                start=(kc == 0),
                                stop=(kc == NDC - 1),
                            )
                        o_sb = otp.tile([P, D], DT, name="osb", tag="osb")
                        nc.vector.tensor_scalar_add(o_sb[:], o_ps[:], bo_t[:, m : m + 1])
                        nc.sync.dma_start(out=outT_v[m][:, ns], in_=o_sb[:])

    nc.compile()
    return nc


def kernel(q, k, v, w_q, b_q, w_k, b_k, w_v, b_v, w_o, b_o):
    global LAST_RESULTS
    key = ("nc", USE_BF16)
    if key not in _compiled:
        _compiled[key] = _build()
    nc = _compiled[key]

    np_dt = ml_dtypes.bfloat16 if USE_BF16 else np.float32

    def packn(x, w):  # [N, 512] -> [N//w, w*512]: w row-chunks side by side
        n = x.shape[0] // (w * P)
        return np.ascontiguousarray(
            x.reshape(n, w, P, D).transpose(0, 2, 1, 3).reshape(n * P, w * D)
        )

    def pack4(x):
        return packn(x, 4)

    q = np.asarray(q, dtype=np.float32)
    kc_ = [packn(np.asarray(k[b], np.float32).astype(np_dt), 2) for b in range(B)]
    vc_ = [packn(np.asarray(v[b], np.float32).astype(np_dt), 2) for b in range(B)]
    wkT = pack4((np.asarray(w_k, np.float32).T * 0.125).astype(np_dt))
    wvT = pack4(np.asarray(w_v, np.float32).T.astype(np_dt))
    wqn = pack4(np.asarray(w_q, np.float32).astype(np_dt))
    woT = pack4(np.asarray(w_o, np.float32).T.astype(np_dt))
    bo = np.ascontiguousarray(np.asarray(b_o, np.float32).reshape(4, P).T)

    in_maps = []
    for c in range(N_CORES):
        b, quarter = divmod(c, 4)
        rows = slice(quarter * SQ, (quarter + 1) * SQ)
        in_maps.append(
            {
                "kb": kc_[b],
                "vb": vc_[b],
                "qT": np.ascontiguousarray(q[b, rows, :].T).astype(np_dt),
                "wkT": wkT,
                "wvT": wvT,
                "wq": wqn,
                "woT": woT,
                "bo": bo,
            }
        )

    res = run_bass_kernel_spmd(nc, in_maps, list(range(N_CORES)), **RUN_KW)
    LAST_RESULTS = res

    out = np.empty((B, S, D), dtype=np.float32)
    for c in range(N_CORES):
        b, quarter = divmod(c, 4)
        rows = slice(quarter * SQ, (quarter + 1) * SQ)
        out[b, rows, :] = res.results[c]["outT"].T.astype(np.float32)
    return out

